# revision 14
# baseline (speedup 1.0000x reference)
"""Trainium2 Bass kernel for nn_PositionalEmbedding (embedding-lookup form).

Math: out[b, 2j]   = mean_k sin(params[k] * dc[b,k] * inv_freq[j])
      out[b, 2j+1] = mean_k cos(params[k] * dc[b,k] * inv_freq[j])

dc[b,k] are integers in [0, 60), so the batch reduction is a one-hot lookup
into a 360-row sin/cos table.  Both factors are built on the HOST:

  *  Tables (a pure function of `params`).  Measured PE law: matmul time =
     streamed moving-operand bytes / 2B-per-cycle @ 2.4 GHz, and only
     DoubleRow fp8 uses the full 2 B/cycle.  Output halves are SPLIT
     ([sin 256 | cos 256], host re-interleaves).  The cos table is
     MEAN-SHIFTED (stores cos-1): fp8's worst absolute steps sit at
     cos ~= 1, so shifting moves the error onto small values and no cos
     residual level is needed (the host adds the 1 back).  Chunk 2 rides
     with a free residual page in the half-used second matmul.
  *  One-hot: fp8 bytes 0x20 (=0.125, folded 8x into the table) at dict row
     60*(2c+kp) + dc[b, 2c+kp]; 5.9 MB/core streamed through SBUF in
     4-group superblocks (6 KB/partition per DMA).  Within each 256-row
     half-group the batch rows are PERMUTED (row 2p+t -> tile t, partition
     p) so each output-DMA descriptor covers 2 consecutive fp8 DRAM rows.

Per output tile the device runs TWO DoubleRow fp8 matmuls (N=512):
     mm1  DR(oh_c0 || oh_c1) x [sin_hi0|cos_hi0 || sin_hi1|cos_hi1]
     mm2  DR(oh_c2 || oh_c2) x [sin_hi2|cos_hi2 || sin_lo2|cos_lo2]
  = 1024 PE cycles/tile (427 ns) - the port-bandwidth floor for this
  precision (1.45e-2 relative error vs the 2e-2 gate, fp8 output included).
  PSUM is organized as 2-bank DUALS (2 tiles) x 4 buffers so the PE runs
  two groups ahead of the drain latency.  Each dual is drained by two
  strided f32->fp8 half-copies (Vector takes sin, Scalar takes cos) and one
  DMA writes fp8 to DRAM (quarter the f32 HBM traffic; host upcasts, adds
  1 to cos, re-interleaves).  Warm-up matmuls on memset-zero weights ramp
  the PE p-state during the prologue; the PE stays the ~100% busy
  bottleneck engine so it holds 2.4 GHz.

Data parallel over 8 NeuronCores: each core handles 16384 rows.
"""

import numpy as np
import ml_dtypes

B = 131072
D = 512
H = D // 2                # 256: one sin/cos half
NCOMP = 6
HYPER = 2100.0
NCORES = 8
BL = B // NCORES          # 16384 rows per core
P = 128                   # partitions / rows per output tile
NV = 60                   # dictionary values per component
CK = 120                  # dictionary rows per chunk (2 components)
NCHUNK = 3
GROUP = 4                 # output tiles per group (512 batch cols)
GCOL = GROUP * P          # 512
SUPER = 2                 # groups per one-hot streaming DMA
TBW = 8 * H               # table bytes per partition: 2 dual pages

NWARM = 6                 # PE warm-up matmuls (p-state ramp during prologue)

_CACHE: dict = {}


def _build_nc(bl, nwarm=NWARM):
    import concourse.bacc as bacc
    import concourse.mybir as mybir
    from concourse import tile

    f32 = mybir.dt.float32
    f8 = mybir.dt.float8e4
    DR = mybir.MatmulPerfMode.DoubleRow

    nc = bacc.Bacc(trn_type="TRN2")
    # one-hot bytes (batch rows 2p+t permuted to one-hot col t*128+p per
    # 256-row half-group): ohd[p, (g*NCHUNK + c)*GCOL + q], dict row p chunk c
    ohd = nc.dram_tensor("ohd", [CK, NCHUNK * bl], f8, kind="ExternalInput").ap()
    # tbd: 2 dual pages: [shi0|chi0|shi1|chi1] [shi2|chi2|slo2|clo2]
    tbd = nc.dram_tensor("tbd", [CK, TBW], f8, kind="ExternalInput").ap()
    out = nc.dram_tensor("out", [bl, D], f8, kind="ExternalOutput").ap()

    ntiles = bl // P
    ngroups = ntiles // GROUP
    nsuper = ngroups // SUPER
    SB = NCHUNK * GCOL * SUPER            # one-hot bytes per superblock row

    with tile.TileContext(nc) as tc:
        with (
            tc.tile_pool(name="const", bufs=1) as cpool,
            tc.tile_pool(name="oh", bufs=6) as ohpool,
            tc.tile_pool(name="osb", bufs=6) as opool,
            tc.tile_pool(name="q", bufs=4, space="PSUM") as qpool,
        ):
            # ---- warm-up weights: memset zeros (no DMA wait)
            wc_sb = cpool.tile([CK, 2 * D], f8, tag="wc")
            nc.vector.memset(wc_sb[:, :], 0.0)

            # ---- constants / first one-hot superblock
            tb_sb = cpool.tile([CK, TBW], f8, tag="tb")
            nc.sync.dma_start(out=tb_sb[:, :], in_=tbd)

            def emit_oh(s, split=1):
                # stream one superblock of one-hot bytes: [120, SB] contiguous
                ohg = ohpool.tile([CK, SUPER, NCHUNK, GCOL], f8, tag="ohg")
                oview = ohg[:, :, :, :].rearrange("p s c j -> p (s c j)")
                step = SB // split
                for i in range(split):
                    # Activation HWDGE ring: keeps one-hot streaming from
                    # queueing behind the output DMAs on the SP ring
                    nc.scalar.dma_start(
                        out=oview[:, i * step:(i + 1) * step],
                        in_=ohd[:, s * SB + i * step:s * SB + (i + 1) * step],
                    )
                return ohg

            ohs = {0: emit_oh(0)}
            for s0 in (1, 2, 3):
                if s0 < nsuper:
                    ohs[s0] = emit_oh(s0)

            rhs1 = tb_sb[:, 0:4 * H].rearrange("p (two n) -> p two n", two=2)
            rhs2 = tb_sb[:, 4 * H:8 * H].rearrange("p (two n) -> p two n", two=2)

            # ---- PE p-state warm-up on zero weights while prologue DMAs land
            wquad = qpool.tile([P, 2, D], f32, tag="q")
            wwt = wc_sb[:, 0:2 * P].rearrange("p (two m) -> p two m", two=2)
            wmv = wc_sb[:, :].rearrange("p (two n) -> p two n", two=2)
            for w in range(nwarm):
                nc.tensor.matmul(
                    wquad[:, w % 2, :], wwt, wmv,
                    start=True, stop=True, perf_mode=DR,
                )

            for g in range(ngroups):
                s, gi = divmod(g, SUPER)
                cur = ohs[s]
                for d in range(2):
                    dual = qpool.tile([P, 2, D], f32, tag="q")
                    for i in range(2):
                        t = 2 * d + i
                        ts = slice(t * P, (t + 1) * P)
                        nc.tensor.matmul(
                            dual[:, i, :], cur[:, gi, 0:2, ts], rhs1,
                            start=True, stop=False, perf_mode=DR,
                        )
                        nc.tensor.matmul(
                            dual[:, i, :],
                            cur[:, gi, 2, ts].unsqueeze(1).broadcast_to([CK, 2, P]),
                            rhs2,
                            start=False, stop=True, perf_mode=DR,
                        )
                    # drain: strided f32->fp8 half-copies (DVE sin, ACT cos)
                    ob = opool.tile([P, 2, D], f8, tag="ob")
                    nc.vector.tensor_copy(out=ob[:, :, 0:H], in_=dual[:, :, 0:H])
                    nc.scalar.copy(out=ob[:, :, H:D], in_=dual[:, :, H:D])
                    r0 = (2 * g + d) * 2 * P
                    dst = out[r0:r0 + 2 * P, :].rearrange(
                        "(p two) j -> p two j", two=2
                    )
                    nc.sync.dma_start(out=dst, in_=ob[:, :, :])
                if gi == 0 and s + 4 < nsuper:
                    ohs[s + 4] = emit_oh(s + 4)
                    if s >= 1:
                        del ohs[s - 1]

    nc.compile()
    return nc


def _get_nc(bl=BL):
    key = ("nc", bl, NWARM)
    if key not in _CACHE:
        _CACHE[key] = _build_nc(bl)
    return _CACHE[key]


def _host_tables(params):
    """fp8 table pages [120, TBW], pre-scaled by 8/6 (0.125 one-hot folded):
    [shi0|chi0|shi1|chi1] [shi2|chi2|slo2|clo2]; cos pages store cos-1."""
    prm = np.asarray(params).astype(np.float32, copy=False).reshape(NCOMP)
    jj = np.arange(0, D, 2, dtype=np.float32)
    inv_freq = (
        np.float32(HYPER) ** (-(np.float32(2.0) * (jj + np.float32(1.0))) / np.float32(D))
    ).astype(np.float32)
    k_idx = np.repeat(np.arange(NCOMP), NV)
    v_idx = np.tile(np.arange(NV), NCOMP).astype(np.float32)
    # same f32 op order as the reference: (param * value) * inv_freq
    ph = (prm[k_idx] * v_idx)[:, None] * inv_freq[None, :]          # [360, 256]
    Ss = (8.0 / NCOMP) * np.sin(ph)
    Yc = (8.0 / NCOMP) * (np.cos(ph) - np.float32(1.0))             # mean-shifted
    f8 = ml_dtypes.float8_e4m3
    Shi = Ss.astype(f8)
    Yhi = Yc.astype(f8)
    Slo = (Ss - Shi.astype(np.float32)).astype(f8)
    Ylo = (Yc - Yhi.astype(np.float32)).astype(f8)

    def rows(c):
        return slice(c * CK, (c + 1) * CK)

    tb = np.zeros((CK, TBW), f8)
    tb[:, 0 * H:1 * H] = Shi[rows(0)]
    tb[:, 1 * H:2 * H] = Yhi[rows(0)]
    tb[:, 2 * H:3 * H] = Shi[rows(1)]
    tb[:, 3 * H:4 * H] = Yhi[rows(1)]
    tb[:, 4 * H:5 * H] = Shi[rows(2)]
    tb[:, 5 * H:6 * H] = Yhi[rows(2)]
    tb[:, 6 * H:7 * H] = Slo[rows(2)]
    tb[:, 7 * H:8 * H] = Ylo[rows(2)]
    return tb


def _host_onehot(dc):
    """fp8 one-hot bytes [NCORES, 120, 3*BL]: 0x20 where
    dc[row, 2c+p//60] == p%60; within each 256-row half-group, batch row
    2p+t is mapped to one-hot column t*128+p (2-row output descriptors)."""
    vals = np.arange(NV, dtype=dc.dtype)
    d = dc.reshape(B, NCHUNK, 2)                      # [i, c, kp]
    oh = np.zeros((2, NV, B, NCHUNK), np.uint8)
    for kp in range(2):
        for c in range(NCHUNK):
            oh[kp, :, :, c] = (d[None, :, c, kp] == vals[:, None]).astype(np.uint8)
    oh *= 0x20                                        # fp8e4m3 0.125
    oh = oh.reshape(2, NV, NCORES, BL // GCOL, GCOL, NCHUNK)
    oh = oh.transpose(2, 0, 1, 3, 5, 4)               # [core, kp, v, g, c, r]
    # permute batch rows r -> one-hot cols q per 256-row half-group:
    # col q (within half-group) holds row 2*(q%128) + q//128
    q = np.arange(GCOL)
    idx = (q // (2 * P)) * 2 * P + 2 * (q % P) + (q % (2 * P)) // P
    oh = oh[..., idx]
    return np.ascontiguousarray(oh).reshape(NCORES, CK, NCHUNK * BL).view(
        ml_dtypes.float8_e4m3
    )


def _in_maps(date_components, params):
    dc = np.asarray(date_components).astype(np.int32, copy=False)
    tb = _host_tables(params)
    oh = _host_onehot(dc)
    return [{"ohd": oh[i], "tbd": tb} for i in range(NCORES)]


def kernel(date_components, params, _trace=False):
    from concourse.bass_utils import run_bass_kernel_spmd

    nc = _get_nc()
    maps = _in_maps(date_components, params)
    res = run_bass_kernel_spmd(
        nc, maps, core_ids=list(range(NCORES)),
        trace=_trace, trace_cores=[0] if _trace else None,
    )
    kernel.last_results = res
    halves = np.concatenate(
        [np.asarray(r["out"]).astype(np.float32) for r in res.results], axis=0
    )
    out = np.empty((B, D), np.float32)
    out[:, 0::2] = halves[:, 0:H]
    out[:, 1::2] = halves[:, H:D] + np.float32(1.0)
    return out


# revision 15
# speedup vs baseline: 1.0091x; 1.0091x over previous
"""Trainium2 Bass kernel for nn_PositionalEmbedding (embedding-lookup form).

Math: out[b, 2j]   = mean_k sin(params[k] * dc[b,k] * inv_freq[j])
      out[b, 2j+1] = mean_k cos(params[k] * dc[b,k] * inv_freq[j])

dc[b,k] are integers in [0, 60), so the batch reduction is a one-hot lookup
into a 360-row sin/cos table.  Both factors are built on the HOST:

  *  Tables (a pure function of `params`).  Measured PE law: matmul time =
     streamed moving-operand bytes / 2B-per-cycle @ 2.4 GHz, and only
     DoubleRow fp8 uses the full 2 B/cycle.  Output halves are SPLIT
     ([sin 256 | cos 256], host re-interleaves).  The cos table is
     MEAN-SHIFTED (stores cos-1): fp8's worst absolute steps sit at
     cos ~= 1, so shifting moves the error onto small values and no cos
     residual level is needed (the host adds the 1 back).  Chunk 2 rides
     with a free residual page in the half-used second matmul.
  *  One-hot: fp8 bytes 0x20 (=0.125, folded 8x into the table) at dict row
     60*(2c+kp) + dc[b, 2c+kp]; 5.9 MB/core streamed through SBUF in
     4-group superblocks (6 KB/partition per DMA).  Within each 256-row
     half-group the batch rows are PERMUTED (row 2p+t -> tile t, partition
     p) so each output-DMA descriptor covers 2 consecutive fp8 DRAM rows.

Per output tile the device runs TWO DoubleRow fp8 matmuls (N=512):
     mm1  DR(oh_c0 || oh_c1) x [sin_hi0|cos_hi0 || sin_hi1|cos_hi1]
     mm2  DR(oh_c2 || oh_c2) x [sin_hi2|cos_hi2 || sin_lo2|cos_lo2]
  = 1024 PE cycles/tile (427 ns) - the port-bandwidth floor for this
  precision (1.45e-2 relative error vs the 2e-2 gate, fp8 output included).
  PSUM is organized as 2-bank DUALS (2 tiles) x 4 buffers so the PE runs
  two groups ahead of the drain latency.  Each dual is drained by two
  strided f32->fp8 half-copies (Vector takes sin, Scalar takes cos) and one
  DMA writes fp8 to DRAM (quarter the f32 HBM traffic; host upcasts, adds
  1 to cos, re-interleaves).  Warm-up matmuls on memset-zero weights ramp
  the PE p-state during the prologue; the PE stays the ~100% busy
  bottleneck engine so it holds 2.4 GHz.

Data parallel over 8 NeuronCores: each core handles 16384 rows.
"""

import numpy as np
import ml_dtypes

B = 131072
D = 512
H = D // 2                # 256: one sin/cos half
NCOMP = 6
HYPER = 2100.0
NCORES = 8
BL = B // NCORES          # 16384 rows per core
P = 128                   # partitions / rows per output tile
NV = 60                   # dictionary values per component
CK = 120                  # dictionary rows per chunk (2 components)
NCHUNK = 3
GROUP = 4                 # output tiles per group (512 batch cols)
GCOL = GROUP * P          # 512
SUPER = 2                 # groups per one-hot streaming DMA
TBW = 8 * H               # table bytes per partition: 2 dual pages

NWARM = 10                # PE warm-up matmuls (p-state ramp during prologue)

_CACHE: dict = {}


def _build_nc(bl, nwarm=NWARM):
    import concourse.bacc as bacc
    import concourse.mybir as mybir
    from concourse import tile

    f32 = mybir.dt.float32
    f8 = mybir.dt.float8e4
    DR = mybir.MatmulPerfMode.DoubleRow

    nc = bacc.Bacc(trn_type="TRN2")
    # one-hot bytes (batch rows 2p+t permuted to one-hot col t*128+p per
    # 256-row half-group): ohd[p, (g*NCHUNK + c)*GCOL + q], dict row p chunk c
    ohd = nc.dram_tensor("ohd", [CK, NCHUNK * bl], f8, kind="ExternalInput").ap()
    # tbd: 2 dual pages: [shi0|chi0|shi1|chi1] [shi2|chi2|slo2|clo2]
    tbd = nc.dram_tensor("tbd", [CK, TBW], f8, kind="ExternalInput").ap()
    out = nc.dram_tensor("out", [bl, D], f8, kind="ExternalOutput").ap()

    ntiles = bl // P
    ngroups = ntiles // GROUP
    nsuper = ngroups // SUPER
    SB = NCHUNK * GCOL * SUPER            # one-hot bytes per superblock row

    with tile.TileContext(nc) as tc:
        with (
            tc.tile_pool(name="const", bufs=1) as cpool,
            tc.tile_pool(name="oh", bufs=6) as ohpool,
            tc.tile_pool(name="osb", bufs=6) as opool,
            tc.tile_pool(name="q", bufs=4, space="PSUM") as qpool,
        ):
            # ---- warm-up weights: memset zeros (no DMA wait)
            wc_sb = cpool.tile([CK, 2 * D], f8, tag="wc")
            nc.vector.memset(wc_sb[:, :], 0.0)

            # ---- constants / first one-hot superblock
            tb_sb = cpool.tile([CK, TBW], f8, tag="tb")
            nc.sync.dma_start(out=tb_sb[:, :], in_=tbd)

            def emit_oh(s, split=1):
                # stream one superblock of one-hot bytes: [120, SB] contiguous
                ohg = ohpool.tile([CK, SUPER, NCHUNK, GCOL], f8, tag="ohg")
                oview = ohg[:, :, :, :].rearrange("p s c j -> p (s c j)")
                step = SB // split
                for i in range(split):
                    # Activation HWDGE ring: keeps one-hot streaming from
                    # queueing behind the output DMAs on the SP ring
                    nc.scalar.dma_start(
                        out=oview[:, i * step:(i + 1) * step],
                        in_=ohd[:, s * SB + i * step:s * SB + (i + 1) * step],
                    )
                return ohg

            ohs = {0: emit_oh(0)}
            for s0 in (1, 2, 3):
                if s0 < nsuper:
                    ohs[s0] = emit_oh(s0)

            rhs1 = tb_sb[:, 0:4 * H].rearrange("p (two n) -> p two n", two=2)
            rhs2 = tb_sb[:, 4 * H:8 * H].rearrange("p (two n) -> p two n", two=2)

            # ---- PE p-state warm-up on zero weights while prologue DMAs land
            wquad = qpool.tile([P, 2, D], f32, tag="q")
            wwt = wc_sb[:, 0:2 * P].rearrange("p (two m) -> p two m", two=2)
            wmv = wc_sb[:, :].rearrange("p (two n) -> p two n", two=2)
            for w in range(nwarm):
                nc.tensor.matmul(
                    wquad[:, w % 2, :], wwt, wmv,
                    start=True, stop=True, perf_mode=DR,
                )

            for g in range(ngroups):
                s, gi = divmod(g, SUPER)
                cur = ohs[s]
                for d in range(2):
                    dual = qpool.tile([P, 2, D], f32, tag="q")
                    for i in range(2):
                        t = 2 * d + i
                        ts = slice(t * P, (t + 1) * P)
                        nc.tensor.matmul(
                            dual[:, i, :], cur[:, gi, 0:2, ts], rhs1,
                            start=True, stop=False, perf_mode=DR,
                        )
                        nc.tensor.matmul(
                            dual[:, i, :],
                            cur[:, gi, 2, ts].unsqueeze(1).broadcast_to([CK, 2, P]),
                            rhs2,
                            start=False, stop=True, perf_mode=DR,
                        )
                    # drain: strided f32->fp8 half-copies (DVE sin, ACT cos)
                    ob = opool.tile([P, 2, D], f8, tag="ob")
                    nc.vector.tensor_copy(out=ob[:, :, 0:H], in_=dual[:, :, 0:H])
                    nc.scalar.copy(out=ob[:, :, H:D], in_=dual[:, :, H:D])
                    r0 = (2 * g + d) * 2 * P
                    dst = out[r0:r0 + 2 * P, :].rearrange(
                        "(p two) j -> p two j", two=2
                    )
                    nc.sync.dma_start(out=dst, in_=ob[:, :, :])
                if gi == 0 and s + 4 < nsuper:
                    ohs[s + 4] = emit_oh(s + 4)
                    if s >= 1:
                        del ohs[s - 1]

    nc.compile()
    return nc


def _get_nc(bl=BL):
    key = ("nc", bl, NWARM)
    if key not in _CACHE:
        _CACHE[key] = _build_nc(bl)
    return _CACHE[key]


def _host_tables(params):
    """fp8 table pages [120, TBW], pre-scaled by 8/6 (0.125 one-hot folded):
    [shi0|chi0|shi1|chi1] [shi2|chi2|slo2|clo2]; cos pages store cos-1."""
    prm = np.asarray(params).astype(np.float32, copy=False).reshape(NCOMP)
    jj = np.arange(0, D, 2, dtype=np.float32)
    inv_freq = (
        np.float32(HYPER) ** (-(np.float32(2.0) * (jj + np.float32(1.0))) / np.float32(D))
    ).astype(np.float32)
    k_idx = np.repeat(np.arange(NCOMP), NV)
    v_idx = np.tile(np.arange(NV), NCOMP).astype(np.float32)
    # same f32 op order as the reference: (param * value) * inv_freq
    ph = (prm[k_idx] * v_idx)[:, None] * inv_freq[None, :]          # [360, 256]
    Ss = (8.0 / NCOMP) * np.sin(ph)
    Yc = (8.0 / NCOMP) * (np.cos(ph) - np.float32(1.0))             # mean-shifted
    f8 = ml_dtypes.float8_e4m3
    Shi = Ss.astype(f8)
    Yhi = Yc.astype(f8)
    Slo = (Ss - Shi.astype(np.float32)).astype(f8)
    Ylo = (Yc - Yhi.astype(np.float32)).astype(f8)

    def rows(c):
        return slice(c * CK, (c + 1) * CK)

    tb = np.zeros((CK, TBW), f8)
    tb[:, 0 * H:1 * H] = Shi[rows(0)]
    tb[:, 1 * H:2 * H] = Yhi[rows(0)]
    tb[:, 2 * H:3 * H] = Shi[rows(1)]
    tb[:, 3 * H:4 * H] = Yhi[rows(1)]
    tb[:, 4 * H:5 * H] = Shi[rows(2)]
    tb[:, 5 * H:6 * H] = Yhi[rows(2)]
    tb[:, 6 * H:7 * H] = Slo[rows(2)]
    tb[:, 7 * H:8 * H] = Ylo[rows(2)]
    return tb


def _host_onehot(dc):
    """fp8 one-hot bytes [NCORES, 120, 3*BL]: 0x20 where
    dc[row, 2c+p//60] == p%60; within each 256-row half-group, batch row
    2p+t is mapped to one-hot column t*128+p (2-row output descriptors)."""
    vals = np.arange(NV, dtype=dc.dtype)
    d = dc.reshape(B, NCHUNK, 2)                      # [i, c, kp]
    oh = np.zeros((2, NV, B, NCHUNK), np.uint8)
    for kp in range(2):
        for c in range(NCHUNK):
            oh[kp, :, :, c] = (d[None, :, c, kp] == vals[:, None]).astype(np.uint8)
    oh *= 0x20                                        # fp8e4m3 0.125
    oh = oh.reshape(2, NV, NCORES, BL // GCOL, GCOL, NCHUNK)
    oh = oh.transpose(2, 0, 1, 3, 5, 4)               # [core, kp, v, g, c, r]
    # permute batch rows r -> one-hot cols q per 256-row half-group:
    # col q (within half-group) holds row 2*(q%128) + q//128
    q = np.arange(GCOL)
    idx = (q // (2 * P)) * 2 * P + 2 * (q % P) + (q % (2 * P)) // P
    oh = oh[..., idx]
    return np.ascontiguousarray(oh).reshape(NCORES, CK, NCHUNK * BL).view(
        ml_dtypes.float8_e4m3
    )


def _in_maps(date_components, params):
    dc = np.asarray(date_components).astype(np.int32, copy=False)
    tb = _host_tables(params)
    oh = _host_onehot(dc)
    return [{"ohd": oh[i], "tbd": tb} for i in range(NCORES)]


def kernel(date_components, params, _trace=False):
    from concourse.bass_utils import run_bass_kernel_spmd

    nc = _get_nc()
    maps = _in_maps(date_components, params)
    res = run_bass_kernel_spmd(
        nc, maps, core_ids=list(range(NCORES)),
        trace=_trace, trace_cores=[0] if _trace else None,
    )
    kernel.last_results = res
    halves = np.concatenate(
        [np.asarray(r["out"]).astype(np.float32) for r in res.results], axis=0
    )
    out = np.empty((B, D), np.float32)
    out[:, 0::2] = halves[:, 0:H]
    out[:, 1::2] = halves[:, H:D] + np.float32(1.0)
    return out
